# revision 28
# baseline (speedup 1.0000x reference)
"""Trainium2 Bass kernel for nn_AreaLoss (topk_masking).

loss = sum(p)/denom + sum(features[b, topk25(main_out[b]), :, :])/denom
with denom = B*H*W. softmax preserves order, so topk on raw logits.

Strategy: data-parallel over batch across 8 cores (8 rows each).
Per core: DVE max8/max_index/match_replace rounds find the top-25
channel indices per row; an indirect DMA gathers ONLY those 200
channel planes (627KB) instead of streaming the full 25MB shard;
reduce to per-partition partials. Host sums partials / denom.
"""

import numpy as np

B, C, H, W = 64, 1000, 28, 28
HW = H * W
NCORES = 8
BLOC = B // NCORES  # 8 batch rows per core
TOPK = 25
NROUNDS = 4  # 4 * 8 = 32 >= 25
DENOM = float(B * H * W)

# gather split: ranks 0-15 (known after round 1, hidden under rounds 2-3)
# -> 128 indices; ranks 16-23 (after round 2) -> 64; rank 24 -> 8, whose
# cast output is already offset-shaped [8,1] (no rearrange DMA).
# (HW indirect DMA consumes ONE index per output partition)
NG = BLOC * TOPK  # 200
KA = 16           # ranks in gather A
GPA = BLOC * KA   # 128
KB = 8            # ranks in gather B
GPB = BLOC * KB   # 64

_CACHE = {}


def _build(dbg=False):
    import concourse.bacc as bacc
    import concourse.tile as tile
    from concourse import mybir
    from concourse.bass import IndirectOffsetOnAxis

    f32 = mybir.dt.float32
    u32 = mybir.dt.uint32

    nc = bacc.Bacc("TRN2", target_bir_lowering=False, debug=False,
                   num_devices=NCORES)
    if dbg:
        dbg_idx = nc.dram_tensor("dbg_idx", [128, 2], u32, kind="ExternalOutput")
        dbg_g = nc.dram_tensor("dbg_g", [128, 2 * HW], f32, kind="ExternalOutput")

    feat = nc.dram_tensor("features", [BLOC * C, HW], f32, kind="ExternalInput")
    mo = nc.dram_tensor("main_out", [BLOC, C], f32, kind="ExternalInput")
    p_in = nc.dram_tensor("p", [BLOC, HW], f32, kind="ExternalInput")
    rowbase = nc.dram_tensor("rowbase", [BLOC, 1], f32, kind="ExternalInput")
    out = nc.dram_tensor("out", [128, 4], f32, kind="ExternalOutput")

    with tile.TileContext(nc) as tc:
        with tc.tile_pool(name="pool", bufs=1) as pool:
            x = pool.tile([BLOC, C], f32, tag="x0")
            nc.sync.dma_start(x[:], mo[:])
            p_t = pool.tile([BLOC, HW], f32, tag="p")
            nc.scalar.dma_start(p_t[:], p_in[:])
            rb = pool.tile([BLOC, 1], f32, tag="rb")
            nc.gpsimd.dma_start(rb[:], rowbase[:])

            out_sb = pool.tile([128, 4], f32, tag="out_sb")
            nc.vector.memset(out_sb[:], 0.0)

            # helper: idx_all[:, c0:c1] + 1000*b -> u32 -> [n*8, 1] offsets.
            # The add+cast runs on the otherwise-idle scalar engine (one
            # activation: u32 out of fp32 Identity with per-partition bias),
            # so it fires the moment the FIND_INDEX8 result lands, without
            # queueing behind the DVE chain.
            def launch_gather(name, c0, c1, parts):
                k = c1 - c0
                idxu = pool.tile([BLOC, k], u32, tag=f"idxu{name}")
                nc.scalar.activation(idxu[:], idx_all[:, c0:c1],
                                     mybir.ActivationFunctionType.Identity,
                                     bias=rb[:, 0:1])
                if k == 1:
                    offs = idxu  # already one offset per partition
                else:
                    offs = pool.tile([parts, 1], u32, tag=f"offs{name}")
                    nc.sync.dma_start(offs[:], idxu[:])
                g = pool.tile([parts, HW], f32, tag=f"g{name}")
                nc.gpsimd.indirect_dma_start(
                    g[:], None, feat[:],
                    IndirectOffsetOnAxis(ap=offs[:], axis=0))
                return offs, g

            # top-32 per row via 4 rounds of max8 + match_replace; gathers
            # A (ranks 0-15) and B (16-23) launch mid-chain and overlap it
            idx_all = pool.tile([BLOC, NROUNDS * 8], u32, tag="idx_all")
            cur = x
            for r in range(NROUNDS):
                vals = pool.tile([BLOC, 8], f32, tag=f"vals{r}")
                nc.vector.max(vals[:], cur[:])
                nc.vector.max_index(idx_all[:, 8 * r:8 * r + 8], vals[:], cur[:])
                if r < NROUNDS - 1:
                    nxt = pool.tile([BLOC, C], f32, tag=f"x{r + 1}")
                    nc.vector.match_replace(nxt[:], vals[:], cur[:], -1e30)
                    cur = nxt
                if r == 1:
                    offsA, gA = launch_gather("A", 0, KA, GPA)
                if r == 2:
                    offsB, gB = launch_gather("B", KA, KA + KB, GPB)

            offsC, gC = launch_gather("C", TOPK - 1, TOPK, BLOC)

            if dbg:
                nc.sync.dma_start(dbg_idx[:, 0:1], offsA[:])
                nc.sync.dma_start(dbg_idx[0:GPB, 1:2], offsB[:])
                nc.sync.dma_start(dbg_g[:, 0:HW], gA[:])
                nc.sync.dma_start(dbg_g[0:GPB, HW:2 * HW], gB[:])

            # reduce gathered planes on DVE (ACT straggles on late data)
            nc.vector.reduce_sum(out_sb[0:GPA, 0:1], gA[:],
                                 axis=mybir.AxisListType.X)
            nc.vector.reduce_sum(out_sb[0:GPB, 1:2], gB[:],
                                 axis=mybir.AxisListType.X)
            nc.vector.reduce_sum(out_sb[0:BLOC, 3:4], gC[:],
                                 axis=mybir.AxisListType.X)

            # sum(p) shard on the scalar engine (off the DVE critical path)
            nc.scalar.activation(p_t[:], p_t[:], mybir.ActivationFunctionType.Copy,
                                 accum_out=out_sb[0:BLOC, 2:3])

            nc.sync.dma_start(out[:], out_sb[:])

    nc.compile()
    return nc


def _get_nc(dbg=False):
    key = ("nc", dbg)
    if key not in _CACHE:
        _CACHE[key] = _build(dbg)
    return _CACHE[key]


def make_in_maps(p, main_out, features):
    p = np.ascontiguousarray(p, dtype=np.float32).reshape(B, HW)
    main_out = np.ascontiguousarray(main_out, dtype=np.float32)
    features = np.ascontiguousarray(features, dtype=np.float32)
    rowbase = (np.arange(BLOC, dtype=np.float32) * C).reshape(BLOC, 1)
    in_maps = []
    for i in range(NCORES):
        sl = slice(i * BLOC, (i + 1) * BLOC)
        in_maps.append({
            "p": p[sl],
            "main_out": main_out[sl],
            "features": features[sl].reshape(BLOC * C, HW),
            "rowbase": rowbase,
        })
    return in_maps


def run_shards(in_maps, trace=False, dbg=False, **kwargs):
    from concourse import bass_utils
    nc = _get_nc(dbg)
    return bass_utils.run_bass_kernel_spmd(
        nc, in_maps, core_ids=list(range(NCORES)), trace=trace, **kwargs)


def kernel(p, main_out, features):
    res = run_shards(make_in_maps(p, main_out, features))
    total = 0.0
    for r in res.results:
        total += r["out"].astype(np.float64).sum()
    return np.asarray(np.float32(total / DENOM))


# revision 33
# speedup vs baseline: 1.1237x; 1.1237x over previous
"""Trainium2 Bass kernel for nn_AreaLoss (topk_masking).

loss = sum(p)/denom + sum(features[b, topk25(main_out[b]), :, :])/denom
with denom = B*H*W. softmax preserves order, so topk on raw logits.

Strategy: data-parallel over batch across 8 cores (8 rows each).
Per core: DVE max8/max_index/match_replace rounds find the top-25
channel indices per row; an indirect DMA gathers ONLY those 200
channel planes (627KB) instead of streaming the full 25MB shard;
reduce to per-partition partials. Host sums partials / denom.
"""

import numpy as np

B, C, H, W = 64, 1000, 28, 28
HW = H * W
NCORES = 8
BLOC = B // NCORES  # 8 batch rows per core
TOPK = 25
NROUNDS = 4  # 4 * 8 = 32 >= 25
DENOM = float(B * H * W)

# gather split: ranks 0-15 (known after round 1, hidden under rounds 2-3)
# -> 128 indices; ranks 16-23 (after round 2) -> 64; rank 24 -> 8, whose
# cast output is already offset-shaped [8,1] (no rearrange DMA).
# (HW indirect DMA consumes ONE index per output partition)
NG = BLOC * TOPK  # 200
KA = 16           # ranks in gather A
GPA = BLOC * KA   # 128
KB = 8            # ranks in gather B
GPB = BLOC * KB   # 64

_CACHE = {}


def _build(dbg=False):
    import concourse.bacc as bacc
    import concourse.tile as tile
    from concourse import mybir
    from concourse.bass import IndirectOffsetOnAxis

    f32 = mybir.dt.float32
    u32 = mybir.dt.uint32

    nc = bacc.Bacc("TRN2", target_bir_lowering=False, debug=False,
                   num_devices=NCORES)
    if dbg:
        dbg_idx = nc.dram_tensor("dbg_idx", [128, 2], u32, kind="ExternalOutput")
        dbg_g = nc.dram_tensor("dbg_g", [128, 2 * HW], f32, kind="ExternalOutput")

    feat = nc.dram_tensor("features", [BLOC * C, HW], f32, kind="ExternalInput")
    mo = nc.dram_tensor("main_out", [BLOC, C], f32, kind="ExternalInput")
    p_in = nc.dram_tensor("p", [BLOC, HW], f32, kind="ExternalInput")
    rowbase = nc.dram_tensor("rowbase", [BLOC, 1], f32, kind="ExternalInput")
    out = nc.dram_tensor("out", [128, 3], f32, kind="ExternalOutput")
    # rank-24 gather dumped raw; host sums it (keeps the last DMA receipt +
    # reduce off the device critical path)
    out2 = nc.dram_tensor("out2", [BLOC, HW], f32, kind="ExternalOutput")

    with tile.TileContext(nc) as tc:
        with tc.tile_pool(name="pool", bufs=1) as pool:
            x = pool.tile([BLOC, C], f32, tag="x0")
            nc.sync.dma_start(x[:, 0:C // 2], mo[:, 0:C // 2])
            nc.scalar.dma_start(x[:, C // 2:], mo[:, C // 2:])
            p_t = pool.tile([BLOC, HW], f32, tag="p")
            nc.scalar.dma_start(p_t[:], p_in[:])
            rb = pool.tile([BLOC, 1], f32, tag="rb")
            nc.gpsimd.dma_start(rb[:], rowbase[:])

            out_sb = pool.tile([128, 3], f32, tag="out_sb")
            nc.vector.memset(out_sb[:], 0.0)

            # helper: idx_all[:, c0:c1] + 1000*b -> u32 -> [n*8, 1] offsets.
            # The add+cast runs on the otherwise-idle scalar engine (one
            # activation: u32 out of fp32 Identity with per-partition bias),
            # so it fires the moment the FIND_INDEX8 result lands, without
            # queueing behind the DVE chain.
            def launch_gather(name, c0, c1, parts):
                k = c1 - c0
                idxu = pool.tile([BLOC, k], u32, tag=f"idxu{name}")
                nc.scalar.activation(idxu[:], idx_all[:, c0:c1],
                                     mybir.ActivationFunctionType.Identity,
                                     bias=rb[:, 0:1])
                if k == 1:
                    offs = idxu  # already one offset per partition
                else:
                    offs = pool.tile([parts, 1], u32, tag=f"offs{name}")
                    nc.sync.dma_start(offs[:], idxu[:])
                g = pool.tile([parts, HW], f32, tag=f"g{name}")
                nc.gpsimd.indirect_dma_start(
                    g[:], None, feat[:],
                    IndirectOffsetOnAxis(ap=offs[:], axis=0))
                return offs, g

            # top-32 per row via 4 rounds of max8 + match_replace; gathers
            # A (ranks 0-15) and B (16-23) launch mid-chain and overlap it
            idx_all = pool.tile([BLOC, NROUNDS * 8], u32, tag="idx_all")
            cur = x
            for r in range(NROUNDS):
                vals = pool.tile([BLOC, 8], f32, tag=f"vals{r}")
                nc.vector.max(vals[:], cur[:])
                nc.vector.max_index(idx_all[:, 8 * r:8 * r + 8], vals[:], cur[:])
                if r < NROUNDS - 1:
                    nxt = pool.tile([BLOC, C], f32, tag=f"x{r + 1}")
                    nc.vector.match_replace(nxt[:], vals[:], cur[:], -1e30)
                    cur = nxt
                if r == 1:
                    offsA, gA = launch_gather("A", 0, KA, GPA)
                if r == 2:
                    offsB, gB = launch_gather("B", KA, KA + KB, GPB)

            offsC, gC = launch_gather("C", TOPK - 1, TOPK, BLOC)

            if dbg:
                nc.sync.dma_start(dbg_idx[:, 0:1], offsA[:])
                nc.sync.dma_start(dbg_idx[0:GPB, 1:2], offsB[:])
                nc.sync.dma_start(dbg_g[:, 0:HW], gA[:])
                nc.sync.dma_start(dbg_g[0:GPB, HW:2 * HW], gB[:])

            # reduce gathered planes on DVE (ACT straggles on late data)
            nc.vector.reduce_sum(out_sb[0:GPA, 0:1], gA[:],
                                 axis=mybir.AxisListType.X)
            nc.vector.reduce_sum(out_sb[0:GPB, 1:2], gB[:],
                                 axis=mybir.AxisListType.X)
            nc.sync.dma_start(out2[:], gC[:])

            # sum(p) shard on the scalar engine (off the DVE critical path)
            nc.scalar.activation(p_t[:], p_t[:], mybir.ActivationFunctionType.Copy,
                                 accum_out=out_sb[0:BLOC, 2:3])

            nc.sync.dma_start(out[:], out_sb[:])

    nc.compile()
    return nc


def _get_nc(dbg=False):
    key = ("nc", dbg)
    if key not in _CACHE:
        _CACHE[key] = _build(dbg)
    return _CACHE[key]


def make_in_maps(p, main_out, features):
    p = np.ascontiguousarray(p, dtype=np.float32).reshape(B, HW)
    main_out = np.ascontiguousarray(main_out, dtype=np.float32)
    features = np.ascontiguousarray(features, dtype=np.float32)
    rowbase = (np.arange(BLOC, dtype=np.float32) * C).reshape(BLOC, 1)
    in_maps = []
    for i in range(NCORES):
        sl = slice(i * BLOC, (i + 1) * BLOC)
        in_maps.append({
            "p": p[sl],
            "main_out": main_out[sl],
            "features": features[sl].reshape(BLOC * C, HW),
            "rowbase": rowbase,
        })
    return in_maps


def run_shards(in_maps, trace=False, dbg=False, **kwargs):
    from concourse import bass_utils
    nc = _get_nc(dbg)
    return bass_utils.run_bass_kernel_spmd(
        nc, in_maps, core_ids=list(range(NCORES)), trace=trace, **kwargs)


def kernel(p, main_out, features):
    res = run_shards(make_in_maps(p, main_out, features))
    total = 0.0
    for r in res.results:
        total += r["out"].astype(np.float64).sum()
        total += r["out2"].astype(np.float64).sum()
    return np.asarray(np.float32(total / DENOM))
